# revision 12
# baseline (speedup 1.0000x reference)
"""GQA attention block (qk-rmsnorm + RoPE + causal GQA attention + out-proj),
tensor-parallel over 8 NeuronCores: 2-way data parallel (batch) x 4-way head
parallel (8 q heads / 2 kv heads per core). All-reduce of out-proj partials is
done on host (sum of 4 partials per batch).

Per-core layouts (device):
  phase 1: q/k/v projections with x^T chunks stationary on PE -> [T,d] rows;
           qk-rmsnorm + RoPE in row layout; PE-transpose q,k to [d,T].
           k^T stored zero-padded per kv slot (kTz0 rows 0:64 = kv0, rows
           64:128 = 0; kTz1 mirrored) so S matmuls run with K=128 stationary
           (K=64 stationaries stream at ~half rate on TRN2).
  phase 2: S^T = kTz_s-chunk.T @ qT-block per head (psum), causal diag mask
           add, exp on ACT (scale=1/8 folded) -> P~ f16; PV via [V|1|0pad]
           stationary (M=128) -> out^T + rowsum in one psum; PV matmuls are
           interleaved between S matmuls so consecutive PE ops never hit the
           same psum bank. Row normalization: DVE reciprocal of the rowsum +
           GpSimd partition_broadcast (no ACT Ln/Exp -> no act-table thrash,
           no PE broadcast matmul, no psum steal from the S pipeline).
  phase 3: out-proj from packed head pairs, f16 weights, accumulate f32 psum.
"""
import sys
import numpy as np

sys.path.insert(0, "/opt/trn_rl_repo")

import concourse.bass as bass  # noqa: E402
import concourse.bacc as bacc  # noqa: E402
import concourse.mybir as mybir  # noqa: E402
import concourse.tile as tile  # noqa: E402
from concourse import masks  # noqa: E402
from concourse.bass_utils import run_bass_kernel_spmd  # noqa: E402

f32 = mybir.dt.float32
f32r = mybir.dt.float32r
f16 = mybir.dt.float16
FT = mybir.ActivationFunctionType
AX = mybir.AxisListType

P = 128
T = 2048
H = 2048
D = 64
NQ = 8          # q heads per core
DQ = NQ * D     # 512
NTT = T // P    # 16 T tiles
NHC = H // P    # 16 hidden chunks
NBLK = 4        # T_q blocks of 512
BLK = 512
EPS = 1e-5
MASKVAL = -30000.0
LN64 = -4.1588830833596715  # ln(1/64): scales exp to keep 1/rowsum in f16 normal range

_CACHE = {}


def _r(ap):
    return ap.bitcast(f32r)


def _build_program():
    nc = bacc.Bacc("TRN2", target_bir_lowering=False, debug=False, num_devices=8)

    xT_d = nc.dram_tensor("xT", [H, T], f16, kind="ExternalInput")
    wqkv_d = nc.dram_tensor("wqkv", [H, 768], f16, kind="ExternalInput")
    wo_d = nc.dram_tensor("wo", [DQ, H], f16, kind="ExternalInput")
    ropeq_d = nc.dram_tensor("ropeq", [T, 128], f16, kind="ExternalInput")
    ropek_d = nc.dram_tensor("ropek", [T, 128], f16, kind="ExternalInput")
    mtab_d = nc.dram_tensor("mtab", [P, 256], f16, kind="ExternalInput")
    out_d = nc.dram_tensor("out", [T, H], f32, kind="ExternalOutput")

    with tile.TileContext(nc) as tc:
        with (
            tc.tile_pool(name="persist", bufs=1) as pp,
            tc.tile_pool(name="work", bufs=2) as wp,
            tc.tile_pool(name="ptp", bufs=4) as ptp,
            tc.tile_pool(name="obp", bufs=8) as obp,
            tc.tile_pool(name="psum", bufs=3, space="PSUM") as ps,
            tc.tile_pool(name="psum_o", bufs=2, space="PSUM") as pso,
        ):
            # ---------- persistent loads (in consumption order) ----------
            # x/wqkv chunks first (the first proj matmul only needs chunk 0),
            # rope/mask tables next (DVE chain at tt=0), wo last (phase 3).
            wqkv_sb = []
            xt_sb = []
            ropeq_sb = pp.tile([P, 16 * 128], f16, tag="ropeq")
            ropek_sb = pp.tile([P, 16 * 128], f16, tag="ropek")
            mtab = pp.tile([P, 256], f16, tag="mtab")
            for hc in range(NHC):
                tx = pp.tile([P, T], f16, tag=f"xt{hc}")
                nc.sync.dma_start(tx[:], xT_d[hc * P:(hc + 1) * P, :])
                xt_sb.append(tx)
                t1 = pp.tile([P, 768], f16, tag=f"wqkv{hc}")
                nc.sync.dma_start(t1[:], wqkv_d[hc * P:(hc + 1) * P, :])
                wqkv_sb.append(t1)
                if hc == 0:
                    # rope/mask tables right after chunk 0: the tt=0 DVE chain
                    # (and the first PE transpose in-stream) needs them early.
                    # one strided descriptor per table, not 16 serial dma_starts
                    nc.sync.dma_start(
                        ropeq_sb[:].rearrange("p (t c) -> p t c", c=128),
                        ropeq_d[:].rearrange("(t p) c -> p t c", p=128))
                    nc.sync.dma_start(
                        ropek_sb[:].rearrange("p (t c) -> p t c", c=128),
                        ropek_d[:].rearrange("(t p) c -> p t c", p=128))
                    nc.sync.dma_start(mtab[:], mtab_d[:])
            wo_sb = []
            for c in range(4):
                t3 = pp.tile([P, H], f16, tag=f"wo{c}")
                nc.sync.dma_start(t3[:], wo_d[c * P:(c + 1) * P, :])
                wo_sb.append(t3)
            negI = mtab[:, 0:128]     # -30000 on diagonal
            ustr = mtab[:, 128:256]   # 1 where k > g (strict lower)
            ident = pp.tile([P, P], f16, tag="ident")
            masks.make_identity(nc, ident[:])
            lnb = pp.tile([P, 1], f32, tag="lnb")
            nc.gpsimd.memset(lnb[:], LN64)
            epsb = pp.tile([P, 1], f32, tag="epsb")
            nc.gpsimd.memset(epsb[:], EPS)

            qT = pp.tile([P, 4 * T], f16, tag="qT")    # chunk c at cols [c*T,(c+1)*T)
            # kTz[s]: zero-padded K^T: rows [s*64,(s+1)*64) = kv-head s dims,
            # other 64 rows stay 0 -> S matmuls get full-K=128 stationaries.
            kTz = []
            for s in range(2):
                kt = pp.tile([P, T], f16, tag=f"kTz{s}")
                nc.gpsimd.memset(kt[:], 0.0)
                kTz.append(kt)
            # vp[tt]: [V_kv0|1|0pad] cols 0:128, [V_kv1|1|0pad] cols 128:256
            # (M=128 stationaries for PV; rows 65:128 of the PV psum become 0)
            vsb = []
            for tt in range(NTT):
                vt = pp.tile([P, 256], f16, tag=f"v{tt}")
                nc.gpsimd.memset(vt[:], 0.0)
                nc.gpsimd.memset(vt[:, 64:65], 1.0)     # ones col for kv0
                nc.gpsimd.memset(vt[:, 192:193], 1.0)   # ones col for kv1
                vsb.append(vt)

            qT3 = qT[:].rearrange("p (c t) -> p c t", t=T)

            # ---------- phase 1: projections + norm + rope + transpose ----------
            pending_tr = []
            for tt in range(NTT):
                pa = ps.tile([P, 1024], f32, tag="a")
                for hc in range(NHC):
                    lhs = xt_sb[hc][:, tt * P:(tt + 1) * P]
                    nc.tensor.matmul(pa[:, 0:512], lhs, wqkv_sb[hc][:, 0:512],
                                     start=(hc == 0), stop=(hc == NHC - 1))
                    nc.tensor.matmul(pa[:, 512:768], lhs, wqkv_sb[hc][:, 512:768],
                                     start=(hc == 0), stop=(hc == NHC - 1))
                # v eviction (no norm): one strided copy into both kv slots
                vt = vsb[tt]
                nc.vector.tensor_copy(
                    vt[:, 0:256].rearrange("p (s c) -> p s c", c=128)[:, :, 0:64],
                    pa[:, 640:768].rearrange("p (s c) -> p s c", c=64))
                # q rmsnorm
                sq = wp.tile([P, DQ], f32, tag="sq")
                nc.scalar.activation(sq[:], pa[:, 0:512], FT.Square)
                red = wp.tile([P, NQ], f32, tag="red")
                nc.vector.reduce_sum(red[:].unsqueeze(-1),
                                     sq[:].rearrange("p (h d) -> p h d", d=D), axis=AX.X)
                srt = wp.tile([P, NQ], f32, tag="srt")
                nc.scalar.activation(srt[:], red[:], FT.Sqrt, scale=1.0 / D, bias=epsb[:])
                rstd = wp.tile([P, NQ], f32, tag="rstd")
                nc.vector.reciprocal_approx_fast(rstd[:], srt[:])
                qn = wp.tile([P, DQ], f32, tag="qn")
                qn3 = qn[:].rearrange("p (h d) -> p h d", d=D)
                nc.vector.tensor_mul(qn3, pa[:, 0:512].rearrange("p (h d) -> p h d", d=D),
                                     rstd[:].unsqueeze(-1).broadcast_to([P, NQ, D]))
                # k rmsnorm
                ksq = wp.tile([P, 128], f32, tag="ksq")
                nc.scalar.activation(ksq[:], pa[:, 512:640], FT.Square)
                kred = wp.tile([P, 2], f32, tag="kred")
                nc.vector.reduce_sum(kred[:].unsqueeze(-1),
                                     ksq[:].rearrange("p (h d) -> p h d", d=D), axis=AX.X)
                ksrt = wp.tile([P, 2], f32, tag="ksrt")
                nc.scalar.activation(ksrt[:], kred[:], FT.Sqrt, scale=1.0 / D, bias=epsb[:])
                krstd = wp.tile([P, 2], f32, tag="krstd")
                nc.vector.reciprocal_approx_fast(krstd[:], ksrt[:])
                kn = wp.tile([P, 128], f32, tag="kn")
                kn3 = kn[:].rearrange("p (h d) -> p h d", d=D)
                nc.vector.tensor_mul(kn3, pa[:, 512:640].rearrange("p (h d) -> p h d", d=D),
                                     krstd[:].unsqueeze(-1).broadcast_to([P, 2, D]))
                # rope q
                cosq = ropeq_sb[:, tt * 128:tt * 128 + 64]
                sinq = ropeq_sb[:, tt * 128 + 64:tt * 128 + 128]
                tcos = wp.tile([P, DQ], f32, tag="tcos")
                nc.vector.tensor_mul(tcos[:].rearrange("p (h d) -> p h d", d=D), qn3,
                                     cosq.unsqueeze(1).broadcast_to([P, NQ, D]))
                rp = wp.tile([P, DQ], f32, tag="rp")
                rp3 = rp[:].rearrange("p (h d) -> p h d", d=D)
                nc.vector.tensor_mul(rp3[:, :, 0:32], qn3[:, :, 32:64],
                                     sinq[:, 0:32].unsqueeze(1).broadcast_to([P, NQ, 32]))
                nc.vector.tensor_mul(rp3[:, :, 32:64], qn3[:, :, 0:32],
                                     sinq[:, 32:64].unsqueeze(1).broadcast_to([P, NQ, 32]))
                qrope = wp.tile([P, DQ], f16, tag="qrope")
                nc.vector.tensor_add(qrope[:], tcos[:], rp[:])
                # rope k
                cosk = ropek_sb[:, tt * 128:tt * 128 + 64]
                sink = ropek_sb[:, tt * 128 + 64:tt * 128 + 128]
                ktcos = wp.tile([P, 128], f32, tag="ktcos")
                nc.vector.tensor_mul(ktcos[:].rearrange("p (h d) -> p h d", d=D), kn3,
                                     cosk.unsqueeze(1).broadcast_to([P, 2, D]))
                krp = wp.tile([P, 128], f32, tag="krp")
                krp3 = krp[:].rearrange("p (h d) -> p h d", d=D)
                nc.vector.tensor_mul(krp3[:, :, 0:32], kn3[:, :, 32:64],
                                     sink[:, 0:32].unsqueeze(1).broadcast_to([P, 2, 32]))
                nc.vector.tensor_mul(krp3[:, :, 32:64], kn3[:, :, 0:32],
                                     sink[:, 32:64].unsqueeze(1).broadcast_to([P, 2, 32]))
                krope = wp.tile([P, 128], f16, tag="krope")
                nc.vector.tensor_add(krope[:], ktcos[:], krp[:])

                # transposes to [d, T] — deferred one tile so PE doesn't stall
                # on this tile's DVE norm/rope chain
                def mk_transp(tt, qrope, krope):
                    def emit():
                        ptr = pso.tile([P, 512], f16, tag="o")
                        for c in range(4):
                            nc.tensor.transpose(ptr[:, c * P:(c + 1) * P],
                                                qrope[:, c * P:(c + 1) * P], ident[:])
                        nc.vector.tensor_copy(qT3[:, :, tt * P:(tt + 1) * P],
                                              ptr[:].rearrange("p (c t) -> p c t", t=P))
                        ptk = pso.tile([P, P], f16, tag="o")
                        nc.tensor.transpose(ptk[:], krope[:], ident[:])
                        nc.vector.tensor_copy(kTz[0][0:64, tt * P:(tt + 1) * P],
                                              ptk[0:64, :])
                        nc.vector.tensor_copy(kTz[1][64:128, tt * P:(tt + 1) * P],
                                              ptk[64:128, :])
                    return emit
                pending_tr.append(mk_transp(tt, qrope, krope))
                if len(pending_tr) > 1:
                    pending_tr.pop(0)()
            for fn in pending_tr:
                fn()

            # ---------- phases 2+3: software-pipelined across pairs/heads ----------
            # PE stream per job: S_jj0(n), [mask], PV_jj0(n-4), S_jj1(n),
            # [mask], PV_jj1(n-4), so consecutive matmuls always alternate
            # psum banks (pa2 bank A / po / pa2 bank B / po).
            DEPTH = 3

            class Unit:
                pass

            def emit_S_half(u, pr, jj, trim_ok):
                i, c, s = u.i, u.c, u.s
                j = 2 * pr + jj
                rel = max(0, (j - 4 * i) * P)
                if not trim_ok:
                    rel = 0
                diag = (j >= 4 * i)
                nc.tensor.matmul(
                    u.pa2[:, jj * 512 + rel:(jj + 1) * 512],
                    kTz[s][:, j * P:(j + 1) * P],
                    qT3[:, c, i * BLK + rel:(i + 1) * BLK],
                    start=True, stop=not diag, skip_group_check=True)
                if diag:
                    mrel = max(0, (j - 4 * i) * P)
                    nc.tensor.matmul(
                        u.pa2[:, jj * 512 + mrel:jj * 512 + mrel + P],
                        negI, ustr, start=False, stop=True,
                        skip_group_check=True)
                return rel

            def emit_exp(u, pr, rels):
                i = u.i
                pt = ptp.tile([P, 1024], f16, tag="pt")
                if pr == 2 * i + 1 and rels[0] > 0:
                    # both halves trimmed: exp only the live spans
                    nc.scalar.activation(pt[:, rels[0]:512],
                                         u.pa2[:, rels[0]:512],
                                         FT.Exp, scale=0.125, bias=lnb[:])
                    nc.scalar.activation(pt[:, 512 + rels[1]:1024],
                                         u.pa2[:, 512 + rels[1]:1024],
                                         FT.Exp, scale=0.125, bias=lnb[:])
                else:
                    nc.scalar.activation(pt[:], u.pa2[:], FT.Exp, scale=0.125,
                                         bias=lnb[:])
                u.pts[pr] = pt
                u.pa2 = None

            def emit_PV_half(u, pr, jj):
                i, s = u.i, u.s
                pt = u.pts[pr]
                j = 2 * pr + jj
                rel = max(0, (j - 4 * i) * P)
                nchunks = 4 * (i + 1)
                nc.tensor.matmul(
                    u.po[:, rel:BLK],
                    vsb[j][:, s * 128:(s + 1) * 128],
                    pt[:, jj * 512 + rel:(jj + 1) * 512],
                    start=(j == 0), stop=(j == nchunks - 1),
                    skip_group_check=True)
                if jj == 1:
                    del u.pts[pr]
                    if pr == nchunks // 2 - 1:
                        pending_norm.append(u)

            def emit_norm(u):
                norms_done[u.i] += 1
                po, s, c = u.po, u.s, u.c
                rs = wp.tile([1, BLK], f32, tag="rs")
                nc.vector.tensor_copy(rs[:], po[64:65, :])
                rv = wp.tile([1, BLK], f32, tag="rv")
                nc.vector.reciprocal_approx_fast(rv[:], rs[:])
                rvh = wp.tile([1, BLK], f16, tag="rvh")
                nc.vector.tensor_copy(rvh[:], rv[:])
                pbs = wp.tile([64, BLK], f16, tag="pbs")
                nc.gpsimd.partition_broadcast(pbs[:], rvh[:])
                if s == 0:
                    nc.vector.tensor_mul(u.ob[0:64, :], po[0:64, :], pbs[:])
                else:
                    scr = wp.tile([64, BLK], f16, tag="scr")
                    nc.vector.tensor_mul(scr[:], po[0:64, :], pbs[:])
                    nc.sync.dma_start(u.ob[64:128, :], scr[:])

            def emit_wo(i, tl, obufs, direct=False):
                tt = i * 4 + tl
                pA0 = ps.tile([P, 1024], f32, tag="a")
                pA1 = ps.tile([P, 1024], f32, tag="a")
                for c in range(4):
                    lhs = obufs[c][:, tl * P:(tl + 1) * P]
                    for h4 in range(4):
                        dst = (pA0 if h4 < 2 else pA1)
                        nc.tensor.matmul(dst[:, (h4 % 2) * 512:(h4 % 2 + 1) * 512],
                                         lhs, wo_sb[c][:, h4 * 512:(h4 + 1) * 512],
                                         start=(c == 0), stop=(c == 3))
                osb0 = wp.tile([P, 1024], f32, tag="osb")
                nc.vector.tensor_copy(osb0[:], pA0[:])
                nc.sync.dma_start(out_d[tt * P:(tt + 1) * P, 0:1024], osb0[:])
                osb1 = wp.tile([P, 1024], f32, tag="osb")
                nc.vector.tensor_copy(osb1[:], pA1[:])
                nc.sync.dma_start(out_d[tt * P:(tt + 1) * P, 1024:2048], osb1[:])

            flat = []      # (unit, pr) S-jobs in emission order
            block_obufs = {}
            for i in range(NBLK):
                block_obufs[i] = {}
                units = []
                for c in range(4):
                    ob = obp.tile([P, BLK], f16, tag="ob")
                    block_obufs[i][c] = ob
                    for s in range(2):
                        u = Unit()
                        u.i, u.c, u.s = i, c, s
                        u.ob = ob
                        u.pts = {}
                        u.pa2 = None
                        u.po = None
                        units.append(u)
                for u in units:
                    for pr in range(2 * (u.i + 1)):
                        flat.append((u, pr, i))

            queue = []     # PV jobs awaiting emission (depth pipeline)
            pending_norm = []
            norms_done = [0] * NBLK
            pending_wo = []
            inject_ctr = 0
            cur_block = 0
            for jobno, (u, pr, i) in enumerate(flat):
                if i != cur_block:
                    for tl in range(4):
                        pending_wo.append((cur_block, tl))
                    cur_block = i
                if u.po is None:
                    u.po = pso.tile([P, BLK], f32, tag="o")
                u.pa2 = ps.tile([P, 1024], f32, tag="a")
                trim_ok = jobno >= 3
                deep = len(queue) > DEPTH
                rel0 = emit_S_half(u, pr, 0, trim_ok)
                if deep:
                    emit_PV_half(*queue[0], 0)
                rel1 = emit_S_half(u, pr, 1, trim_ok)
                if deep:
                    emit_PV_half(*queue.pop(0), 1)
                emit_exp(u, pr, (rel0, rel1))
                queue.append((u, pr))
                # deferred normalizes: at most one per S-job
                if pending_norm:
                    emit_norm(pending_norm.pop(0))
                # inject one deferred wo-task every 6 S-jobs once its block's
                # normalizes have all been emitted
                inject_ctr += 1
                if (pending_wo and inject_ctr % 6 == 0
                        and norms_done[pending_wo[0][0]] == 8):
                    wb, tl = pending_wo.pop(0)
                    emit_wo(wb, tl, block_obufs[wb])
            while queue:
                emit_PV_half(*queue[0], 0)
                emit_PV_half(*queue.pop(0), 1)
            for u in pending_norm:
                emit_norm(u)
            pending_norm = []
            for tl in range(4):
                pending_wo.append((NBLK - 1, tl))
            for wb, tl in pending_wo:
                emit_wo(wb, tl, block_obufs[wb], direct=(wb == NBLK - 1))

    nc.compile()
    return nc


def _host_inputs(x, Wq, Wk, Wv, Wo, q_ln_w, k_ln_w):
    x = np.asarray(x, np.float32)
    Wq = np.asarray(Wq, np.float32)
    Wk = np.asarray(Wk, np.float32)
    Wv = np.asarray(Wv, np.float32)
    Wo = np.asarray(Wo, np.float32)
    q_ln_w = np.asarray(q_ln_w, np.float64)
    k_ln_w = np.asarray(k_ln_w, np.float64)

    inv_freq = 1.0 / (1e6 ** (np.arange(0, D, 2, dtype=np.float64) / D))
    t = np.arange(T, dtype=np.float64)
    freqs = np.outer(t, inv_freq)
    emb = np.concatenate([freqs, freqs], -1)
    cos, sin = np.cos(emb), np.sin(emb)
    rot = (np.arange(D) + 32) % D
    sign = np.where(np.arange(D) < 32, -1.0, 1.0)

    def rope_tab(w):
        cw = w[None, :] * cos
        sw = sign[None, :] * w[rot][None, :] * sin
        return np.concatenate([cw, sw], -1).astype(np.float16)

    ropeq = rope_tab(q_ln_w)
    ropek = rope_tab(k_ln_w)
    pp_, gg_ = np.meshgrid(np.arange(P), np.arange(P), indexing="ij")
    negI = np.where(pp_ == gg_, MASKVAL, 0.0)
    ustr = (pp_ > gg_).astype(np.float64)
    mtab = np.concatenate([negI, ustr], axis=1).astype(np.float16)

    in_maps = []
    for core in range(8):
        b, g = core // 4, core % 4
        xT = np.ascontiguousarray(x[b].T).astype(np.float16)
        heads = []
        for c in range(4):
            heads += [g * 8 + c, g * 8 + c + 4]
        wqkv = np.ascontiguousarray(np.concatenate(
            [Wq[:, h * D:(h + 1) * D] for h in heads]
            + [Wk[:, g * 128:(g + 1) * 128], Wv[:, g * 128:(g + 1) * 128]],
            axis=1)).astype(np.float16)
        wo = np.ascontiguousarray(
            np.concatenate([Wo[h * D:(h + 1) * D, :] for h in heads], axis=0)
        ).astype(np.float16)
        in_maps.append({
            "xT": xT, "wqkv": wqkv, "wo": wo,
            "ropeq": ropeq, "ropek": ropek, "mtab": mtab,
        })
    return in_maps


def get_program():
    if "nc" not in _CACHE:
        _CACHE["nc"] = _build_program()
    return _CACHE["nc"]


def run(inputs, trace=False, tmpdir=None):
    nc = get_program()
    in_maps = _host_inputs(**inputs)
    res = run_bass_kernel_spmd(nc, in_maps, list(range(8)), trace=trace, tmpdir=tmpdir)
    out = np.zeros((2, T, H), np.float32)
    for core in range(8):
        out[core // 4] += res.results[core]["out"]
    return out, res


def kernel(**inputs) -> np.ndarray:
    out, _ = run(inputs, trace=False)
    return out
